# revision 2
# baseline (speedup 1.0000x reference)
"""Bass/Trainium2 kernel for nn_Attention (Bahdanau-style attention scores).

reference:
    h = hidden[0]                               # (B, H)
    e = encoder_outputs.swapaxes(0, 1)          # (B, S, H)
    energy = tanh(e @ We.T + h @ Wh.T + b)      # (B, S, H)
    scores = energy @ v                         # (B, S)
    out = softmax(scores, axis=1)[:, None, :]   # (B, 1, S)

Strategy: data-parallel over batch B=32 across 8 cores (4 batches/core,
no collectives). Per core, layout [k partitions, s free]:
  - main matmul: pre_e[k, s] += WeT[h, k].T @ eT[h, s], accumulated over
    8 h-tiles into PSUM (bf16 inputs, fp32 accumulation).
  - bias (h @ Wh.T + b) is computed once per core as per-partition column
    vectors and fused into the ACT-engine tanh.
  - v-dot is a K=128, M=1 matmul accumulating scores into PSUM over k-tiles.
  - softmax over S without max-subtraction (scores are O(1), exp is safe in
    fp32): chunked exp straight out of PSUM with fused accumulate, combine
    sums, reciprocal, chunked scale.
Host side pre-transposes W/encoder_outputs (bf16) so all device DMAs are
coalesced; output is fp32.
"""
import numpy as np

S, B, H = 2048, 32, 1024
NCORES = 8
BPC = B // NCORES           # batches per core = 4
KT = H // 128               # 8 k-tiles (output dim of We)
HT = H // 128               # 8 h-tiles (contraction dim)
HB = H + 128                # 1152 = padded contraction for [Wh | b] with ones row
HBT = HB // 128             # 9
NSC = 4                     # s-chunks per batch
SC = S // NSC               # 512

_cache = {}


def _build():
    import concourse.tile as tile
    from concourse import bacc, mybir

    f32 = mybir.dt.float32
    bf16 = mybir.dt.bfloat16
    Tanh = mybir.ActivationFunctionType.Tanh
    Exp = mybir.ActivationFunctionType.Exp

    nc = bacc.Bacc("TRN2", target_bir_lowering=False, debug=False,
                   num_devices=NCORES)

    eT_d = nc.dram_tensor("eT", [BPC, H, S], bf16, kind="ExternalInput").ap()
    WeT_d = nc.dram_tensor("WeT", [H, H], bf16, kind="ExternalInput").ap()
    WhbT_d = nc.dram_tensor("WhbT", [HB, H], bf16, kind="ExternalInput").ap()
    hT_d = nc.dram_tensor("hT", [HB, BPC], bf16, kind="ExternalInput").ap()
    v_d = nc.dram_tensor("v", [KT, 128, 1], bf16, kind="ExternalInput").ap()
    out_d = nc.dram_tensor("out", [BPC, S], f32, kind="ExternalOutput").ap()

    with tile.TileContext(nc) as tc:
        with (
            tc.tile_pool(name="w", bufs=1) as wpool,
            tc.tile_pool(name="e", bufs=2 * HT) as epool,
            tc.tile_pool(name="en", bufs=8) as enpool,
            tc.tile_pool(name="sm", bufs=2) as spool,
            tc.tile_pool(name="pm", bufs=4, space="PSUM") as pmpool,
            tc.tile_pool(name="pv", bufs=4, space="PSUM") as pvpool,
        ):
            # ---- tiny inputs first (hT, v), then interleaved streams so the
            # first main matmuls and the bias matmuls can start ASAP ----
            hT_sb = []
            for ht in range(HBT):
                t = wpool.tile([128, BPC], bf16, tag=f"hT{ht}")
                nc.sync.dma_start(t[:], hT_d[ht * 128:(ht + 1) * 128, :])
                hT_sb.append(t)
            v_sb = []
            for kt in range(KT):
                t = wpool.tile([128, 1], bf16, tag=f"v{kt}")
                nc.sync.dma_start(t[:], v_d[kt])
                v_sb.append(t)

            WeT_sb = []
            WhbT_sb = []
            e_sb0 = []
            for ht in range(HT):
                t = wpool.tile([128, H], bf16, tag=f"WeT{ht}")
                nc.sync.dma_start(t[:], WeT_d[ht * 128:(ht + 1) * 128, :])
                WeT_sb.append(t)
                t = epool.tile([128, S], bf16, tag="e", name="e_t")
                nc.sync.dma_start(t[:], eT_d[0, ht * 128:(ht + 1) * 128, :])
                e_sb0.append(t)
                t = wpool.tile([128, H], bf16, tag=f"WhbT{ht}")
                nc.sync.dma_start(t[:], WhbT_d[ht * 128:(ht + 1) * 128, :])
                WhbT_sb.append(t)
            t = wpool.tile([128, H], bf16, tag=f"WhbT{HBT - 1}")
            nc.sync.dma_start(t[:], WhbT_d[(HBT - 1) * 128:HBT * 128, :])
            WhbT_sb.append(t)

            # ---- bias = hidden @ Wh.T + b, laid out [k partitions, b free];
            # one PSUM tile [128, KT*BPC], kt-slices within one zero region ----
            ph = pmpool.tile([128, KT * BPC], f32, tag="mp")
            for ht in range(HBT):
                for kt in range(KT):
                    nc.tensor.matmul(
                        ph[:, kt * BPC:(kt + 1) * BPC],
                        lhsT=WhbT_sb[ht][:, kt * 128:(kt + 1) * 128],
                        rhs=hT_sb[ht][:],
                        start=(ht == 0 and kt == 0),
                        stop=(ht == HBT - 1 and kt == KT - 1),
                    )
            bias_sb = wpool.tile([128, KT * BPC], f32, tag="bias")
            nc.vector.tensor_copy(bias_sb[:], ph[:])

            # ---- main loop over batches ----
            for b in range(BPC):
                if b == 0:
                    e_sb = e_sb0
                else:
                    e_sb = []
                    for ht in range(HT):
                        t = epool.tile([128, S], bf16, tag="e", name="e_t")
                        nc.sync.dma_start(t[:], eT_d[b, ht * 128:(ht + 1) * 128, :])
                        e_sb.append(t)
                vps = [pvpool.tile([1, SC], f32, tag="vp", name="vps")
                       for _ in range(NSC)]
                for kt in range(KT):
                    mps = [pmpool.tile([128, SC], f32, tag="mp", name="mps")
                           for _ in range(NSC)]
                    for ht in range(HT):
                        for sc in range(NSC):
                            nc.tensor.matmul(
                                mps[sc][:],
                                lhsT=WeT_sb[ht][:, kt * 128:(kt + 1) * 128],
                                rhs=e_sb[ht][:, sc * SC:(sc + 1) * SC],
                                start=(ht == 0), stop=(ht == HT - 1),
                            )
                    ens = []
                    for sc in range(NSC):
                        en = enpool.tile([128, SC], bf16, tag="en", name="en")
                        nc.scalar.activation(en[:], mps[sc][:], Tanh,
                                             bias=bias_sb[:, kt * BPC + b:
                                                          kt * BPC + b + 1])
                        ens.append(en)
                    for sc in range(NSC):
                        nc.tensor.matmul(
                            vps[sc][:], lhsT=v_sb[kt][:], rhs=ens[sc][:],
                            start=(kt == 0), stop=(kt == KT - 1),
                        )
                # ---- softmax over S (no max subtraction; scores are O(1)) ----
                ex = spool.tile([1, S], f32, tag="exp")
                ssums = spool.tile([1, NSC], f32, tag="ssums")
                for sc in range(NSC):
                    nc.scalar.activation(ex[:, sc * SC:(sc + 1) * SC], vps[sc][:],
                                         Exp, accum_out=ssums[:, sc:sc + 1])
                stot = spool.tile([1, 1], f32, tag="stot")
                nc.vector.tensor_reduce(stot[:], ssums[:],
                                        axis=mybir.AxisListType.X,
                                        op=mybir.AluOpType.add)
                rec = spool.tile([1, 1], f32, tag="rec")
                nc.vector.reciprocal(rec[:], stot[:])
                ot = spool.tile([1, S], f32, tag="ot")
                for sc in range(NSC):
                    nc.vector.tensor_scalar_mul(ot[:, sc * SC:(sc + 1) * SC],
                                                ex[:, sc * SC:(sc + 1) * SC],
                                                rec[:])
                nc.sync.dma_start(out_d[b:b + 1, :], ot[:])

    nc.compile()
    return nc


def _prep_inputs(hidden, encoder_outputs, W, b, v):
    import ml_dtypes
    bf16 = ml_dtypes.bfloat16

    hidden = np.asarray(hidden, dtype=np.float32)
    encoder_outputs = np.asarray(encoder_outputs, dtype=np.float32)
    W = np.asarray(W, dtype=np.float32)
    b = np.asarray(b, dtype=np.float32)
    v = np.asarray(v, dtype=np.float32)

    # (S, B, H) -> (B, H, S) in bf16; per-core slices are contiguous views
    eT_all = np.ascontiguousarray(encoder_outputs.transpose(1, 2, 0)).astype(bf16)
    W_bf = W.astype(bf16)
    WhT = np.ascontiguousarray(W_bf[:, :H].T)          # [h, k]
    WeT = np.ascontiguousarray(W_bf[:, H:].T)          # [h, k]
    WhbT = np.concatenate(
        [WhT, b.astype(bf16)[None, :], np.zeros((127, H), dtype=bf16)], axis=0)
    h_bf = hidden[0].astype(bf16)                      # (B, H)
    v_t = v.astype(bf16).reshape(KT, 128, 1)

    in_maps = []
    for i in range(NCORES):
        sl = slice(i * BPC, (i + 1) * BPC)
        hT = np.concatenate(
            [np.ascontiguousarray(h_bf[sl].T),
             np.ones((1, BPC), dtype=bf16),
             np.zeros((127, BPC), dtype=bf16)], axis=0)
        in_maps.append({
            "eT": eT_all[sl],
            "WeT": WeT,
            "WhbT": WhbT,
            "hT": hT,
            "v": v_t,
        })
    return in_maps


def kernel_with_results(hidden, encoder_outputs, W, b, v):
    from concourse.bass_utils import run_bass_kernel_spmd

    if "nc" not in _cache:
        _cache["nc"] = _build()
    nc = _cache["nc"]
    in_maps = _prep_inputs(hidden, encoder_outputs, W, b, v)
    res = run_bass_kernel_spmd(nc, in_maps, core_ids=list(range(NCORES)))
    out = np.concatenate([res.results[i]["out"] for i in range(NCORES)], axis=0)
    return out[:, None, :].astype(np.float32), res


def kernel(hidden, encoder_outputs, W, b, v):
    out, _ = kernel_with_results(hidden, encoder_outputs, W, b, v)
    return out


# revision 4
# speedup vs baseline: 1.1021x; 1.1021x over previous
"""Bass/Trainium2 kernel for nn_Attention (Bahdanau-style attention scores).

reference:
    h = hidden[0]                               # (B, H)
    e = encoder_outputs.swapaxes(0, 1)          # (B, S, H)
    energy = tanh(e @ We.T + h @ Wh.T + b)      # (B, S, H)
    scores = energy @ v                         # (B, S)
    out = softmax(scores, axis=1)[:, None, :]   # (B, 1, S)

Strategy: data-parallel over batch B=32 across 8 cores (4 batches/core,
no collectives). Per core, layout [k partitions, s free]:
  - main matmul: pre_e[k, s] += WeT[h, k].T @ eT[h, s], accumulated over
    8 h-tiles into PSUM (bf16 inputs, fp32 accumulation).
  - bias (h @ Wh.T + b) is computed once per core as per-partition column
    vectors and fused into the ACT-engine tanh.
  - v-dot is a K=128, M=1 matmul accumulating scores into PSUM over k-tiles.
  - softmax over S without max-subtraction (scores are O(1), exp is safe in
    fp32): chunked exp straight out of PSUM with fused accumulate, combine
    sums, reciprocal, chunked scale.
Host side pre-transposes W/encoder_outputs (bf16) so all device DMAs are
coalesced; output is fp32.
"""
import numpy as np

S, B, H = 2048, 32, 1024
NCORES = 8
BPC = B // NCORES           # batches per core = 4
KT = H // 128               # 8 k-tiles (output dim of We)
HT = H // 128               # 8 h-tiles (contraction dim)
HB = H + 128                # 1152 = padded contraction for [Wh | b] with ones row
HBT = HB // 128             # 9
NSC = 4                     # s-chunks per batch
SC = S // NSC               # 512

_cache = {}


def _build():
    import concourse.tile as tile
    from concourse import bacc, mybir

    f32 = mybir.dt.float32
    bf16 = mybir.dt.bfloat16
    Tanh = mybir.ActivationFunctionType.Tanh
    Exp = mybir.ActivationFunctionType.Exp

    nc = bacc.Bacc("TRN2", target_bir_lowering=False, debug=False,
                   num_devices=NCORES)

    eT_d = nc.dram_tensor("eT", [BPC, H, S], bf16, kind="ExternalInput").ap()
    WeT_d = nc.dram_tensor("WeT", [H, H], bf16, kind="ExternalInput").ap()
    WhbT_d = nc.dram_tensor("WhbT", [HB, H], bf16, kind="ExternalInput").ap()
    hT_d = nc.dram_tensor("hT", [HB, BPC], bf16, kind="ExternalInput").ap()
    v_d = nc.dram_tensor("v", [KT, 128, 1], bf16, kind="ExternalInput").ap()
    out_d = nc.dram_tensor("out", [BPC, S], f32, kind="ExternalOutput").ap()

    with tile.TileContext(nc) as tc:
        with (
            tc.tile_pool(name="w", bufs=1) as wpool,
            tc.tile_pool(name="e", bufs=2 * HT) as epool,
            tc.tile_pool(name="en", bufs=8) as enpool,
            tc.tile_pool(name="sm", bufs=2) as spool,
            tc.tile_pool(name="pm", bufs=4, space="PSUM") as pmpool,
            tc.tile_pool(name="pv", bufs=4, space="PSUM") as pvpool,
        ):
            # ---- DMA priority order: bias inputs first (small, unblock the
            # PE-first prehb matmuls), then e0 + the kt0 column of WeT (what
            # the first main matmuls need), then the remaining WeT columns ----
            hT_sb = []
            for ht in range(HBT):
                t = wpool.tile([128, BPC], bf16, tag=f"hT{ht}")
                nc.sync.dma_start(t[:], hT_d[ht * 128:(ht + 1) * 128, :])
                hT_sb.append(t)
            v_sb = []
            for kt in range(KT):
                t = wpool.tile([128, 1], bf16, tag=f"v{kt}")
                nc.sync.dma_start(t[:], v_d[kt])
                v_sb.append(t)
            WhbT_sb = []
            for ht in range(HBT):
                t = wpool.tile([128, H], bf16, tag=f"WhbT{ht}")
                nc.sync.dma_start(t[:], WhbT_d[ht * 128:(ht + 1) * 128, :])
                WhbT_sb.append(t)

            # WeT as 64 column tiles [128(h), 128(k)] so kt=0 only waits on
            # 8 small tiles instead of the full 2MB
            WeT_sb = [[None] * KT for _ in range(HT)]
            e_sb0 = []
            for ht in range(HT):
                t = epool.tile([128, S], bf16, tag="e", name="e_t")
                nc.sync.dma_start(t[:], eT_d[0, ht * 128:(ht + 1) * 128, :])
                e_sb0.append(t)
                t = wpool.tile([128, 128], bf16, tag=f"WeT{ht}_0", name="WeT_t")
                nc.sync.dma_start(t[:], WeT_d[ht * 128:(ht + 1) * 128, 0:128])
                WeT_sb[ht][0] = t
            for kt in range(1, KT):
                for ht in range(HT):
                    t = wpool.tile([128, 128], bf16, tag=f"WeT{ht}_{kt}",
                                   name="WeT_t")
                    nc.sync.dma_start(
                        t[:], WeT_d[ht * 128:(ht + 1) * 128,
                                    kt * 128:(kt + 1) * 128])
                    WeT_sb[ht][kt] = t

            # ---- bias = hidden @ Wh.T + b, laid out [k partitions, b free];
            # one PSUM tile [128, KT*BPC], kt-slices within one zero region ----
            ph = pvpool.tile([128, KT * BPC], f32, tag="vp")
            for ht in range(HBT):
                for kt in range(KT):
                    nc.tensor.matmul(
                        ph[:, kt * BPC:(kt + 1) * BPC],
                        lhsT=WhbT_sb[ht][:, kt * 128:(kt + 1) * 128],
                        rhs=hT_sb[ht][:],
                        start=(ht == 0 and kt == 0),
                        stop=(ht == HBT - 1 and kt == KT - 1),
                    )
            bias_sb = wpool.tile([128, KT * BPC], f32, tag="bias")
            nc.vector.tensor_copy(bias_sb[:], ph[:])

            # ---- main loop over batches ----
            for b in range(BPC):
                if b == 0:
                    e_sb = e_sb0
                else:
                    e_sb = []
                    for ht in range(HT):
                        t = epool.tile([128, S], bf16, tag="e", name="e_t")
                        nc.sync.dma_start(t[:], eT_d[b, ht * 128:(ht + 1) * 128, :])
                        e_sb.append(t)
                vps = [pvpool.tile([1, SC], f32, tag="vp", name="vps")
                       for _ in range(NSC)]
                # v-dots are deferred by one kt so they never wait on a
                # just-finished tanh (deferred dep = no PE pipeline drain)
                pend = None
                for kt in range(KT):
                    mps = [pmpool.tile([128, SC], f32, tag="mp", name="mps")
                           for _ in range(NSC)]
                    for ht in range(HT):
                        for sc in range(NSC):
                            nc.tensor.matmul(
                                mps[sc][:],
                                lhsT=WeT_sb[ht][kt][:],
                                rhs=e_sb[ht][:, sc * SC:(sc + 1) * SC],
                                start=(ht == 0), stop=(ht == HT - 1),
                            )
                    ens = []
                    for sc in range(NSC):
                        en = enpool.tile([128, SC], bf16, tag="en", name="en")
                        nc.scalar.activation(en[:], mps[sc][:], Tanh,
                                             bias=bias_sb[:, kt * BPC + b:
                                                          kt * BPC + b + 1])
                        ens.append(en)
                    if pend is not None:
                        for sc in range(NSC):
                            nc.tensor.matmul(
                                vps[sc][:], lhsT=v_sb[kt - 1][:],
                                rhs=pend[sc][:],
                                start=(kt - 1 == 0), stop=False,
                            )
                    pend = ens
                for sc in range(NSC):
                    nc.tensor.matmul(
                        vps[sc][:], lhsT=v_sb[KT - 1][:], rhs=pend[sc][:],
                        start=False, stop=True,
                    )
                # ---- softmax over S (no max subtraction; scores are O(1)) ----
                ex = spool.tile([1, S], f32, tag="exp")
                ssums = spool.tile([1, NSC], f32, tag="ssums")
                for sc in range(NSC):
                    nc.scalar.activation(ex[:, sc * SC:(sc + 1) * SC], vps[sc][:],
                                         Exp, accum_out=ssums[:, sc:sc + 1])
                stot = spool.tile([1, 1], f32, tag="stot")
                nc.vector.tensor_reduce(stot[:], ssums[:],
                                        axis=mybir.AxisListType.X,
                                        op=mybir.AluOpType.add)
                rec = spool.tile([1, 1], f32, tag="rec")
                nc.vector.reciprocal(rec[:], stot[:])
                ot = spool.tile([1, S], f32, tag="ot")
                for sc in range(NSC):
                    nc.vector.tensor_scalar_mul(ot[:, sc * SC:(sc + 1) * SC],
                                                ex[:, sc * SC:(sc + 1) * SC],
                                                rec[:])
                nc.sync.dma_start(out_d[b:b + 1, :], ot[:])

    nc.compile()
    return nc


def _prep_inputs(hidden, encoder_outputs, W, b, v):
    import ml_dtypes
    bf16 = ml_dtypes.bfloat16

    hidden = np.asarray(hidden, dtype=np.float32)
    encoder_outputs = np.asarray(encoder_outputs, dtype=np.float32)
    W = np.asarray(W, dtype=np.float32)
    b = np.asarray(b, dtype=np.float32)
    v = np.asarray(v, dtype=np.float32)

    # (S, B, H) -> (B, H, S) in bf16; per-core slices are contiguous views
    eT_all = np.ascontiguousarray(encoder_outputs.transpose(1, 2, 0)).astype(bf16)
    W_bf = W.astype(bf16)
    WhT = np.ascontiguousarray(W_bf[:, :H].T)          # [h, k]
    WeT = np.ascontiguousarray(W_bf[:, H:].T)          # [h, k]
    WhbT = np.concatenate(
        [WhT, b.astype(bf16)[None, :], np.zeros((127, H), dtype=bf16)], axis=0)
    h_bf = hidden[0].astype(bf16)                      # (B, H)
    v_t = v.astype(bf16).reshape(KT, 128, 1)

    in_maps = []
    for i in range(NCORES):
        sl = slice(i * BPC, (i + 1) * BPC)
        hT = np.concatenate(
            [np.ascontiguousarray(h_bf[sl].T),
             np.ones((1, BPC), dtype=bf16),
             np.zeros((127, BPC), dtype=bf16)], axis=0)
        in_maps.append({
            "eT": eT_all[sl],
            "WeT": WeT,
            "WhbT": WhbT,
            "hT": hT,
            "v": v_t,
        })
    return in_maps


def kernel_with_results(hidden, encoder_outputs, W, b, v):
    from concourse.bass_utils import run_bass_kernel_spmd

    if "nc" not in _cache:
        _cache["nc"] = _build()
    nc = _cache["nc"]
    in_maps = _prep_inputs(hidden, encoder_outputs, W, b, v)
    res = run_bass_kernel_spmd(nc, in_maps, core_ids=list(range(NCORES)))
    out = np.concatenate([res.results[i]["out"] for i in range(NCORES)], axis=0)
    return out[:, None, :].astype(np.float32), res


def kernel(hidden, encoder_outputs, W, b, v):
    out, _ = kernel_with_results(hidden, encoder_outputs, W, b, v)
    return out
